# revision 25
# baseline (speedup 1.0000x reference)
"""CrossAttention Trainium2 kernel (8 NeuronCores, SPMD, no collectives).

Problem: nn_CrossAttention_1563368096520
  hidden_states [2, 4096, 512], encoder_hidden_states [2, 4096, 768]
  w_q [512,512], w_k/w_v [768,512], w_out [512,512], b_out [512]
  out = softmax((hs@w_q) @ (enc@w_k)^T * dh^-0.5) @ (enc@w_v) @ w_out + b_out
  (8 heads of dim 64)

Sharding: q-rows. Core c handles batch b=c//4, query rows [(c%4)*1024,
(c%4+1)*1024). Each core recomputes K/V projections for its batch (4x
duplicated) which avoids all cross-core communication.

Layout strategy (bf16 matmuls, fp32 PSUM/softmax-denominator/output):
  - Host pre-transposes hs and enc so the kernel needs no input transposes.
  - qT/kT [inner, rows] come straight out of the projections with w_q/w_k
    as the stationary operand.
  - scores are computed transposed (k on partitions, q free): the exp'd
    score tiles are directly the rhs of the PV matmul - no probs transpose.
  - scores pre-scaled by dh^-0.5 via w_q (exact: *0.125 in bf16).
  - softmax skips max-subtraction (scores in [-2.1, 2.1] for this problem's
    distribution; softmax is shift invariant, exp is exact there).
  - PV runs "flipped": acc[65, q] += v_aug[k,65].T @ expT[k,q] with a ones
    column in v_aug producing the softmax denominator row for free.
    512 N=512 matmuls instead of 2048 N=65 ones.
  - Per head the acc is PE-transposed back to q-partition layout where the
    denominator is a column: reciprocal + tensor_scalar broadcast.
  - K/V/Q projections are interleaved into the first heads' score/exp
    stream so the PE warms up while ACT (the exp bottleneck) streams.
"""

import sys
from contextlib import ExitStack

for _p in ("/opt/trn_rl_repo", "/opt/pypackages"):
    if _p not in sys.path:
        sys.path.append(_p)

import numpy as np
import ml_dtypes

import concourse.bass as bass  # noqa: F401
import concourse.tile as tile
from concourse import bacc, mybir
from concourse.bass_utils import run_bass_kernel_spmd
from concourse.masks import make_identity

BF16 = mybir.dt.bfloat16
F32 = mybir.dt.float32
NPBF16 = ml_dtypes.bfloat16

B, SQ, SKV = 2, 4096, 4096
QD, CD = 512, 768
H, DH = 8, 64
INNER = H * DH  # 512
SCALE = DH ** -0.5
NCORES = 8
QR = (B * SQ) // NCORES  # 1024 query rows per core
QCH = QR // 128          # 8 q chunks per core
KCH = SKV // 128         # 32 kv chunks
DHA = DH + 1             # v columns + ones column

_cache: dict = {}


def _emit(nc, tc, ctx, hsT_d, encT_d, wq_d, wk_d, wv_d, wo_d, bias_d, out_d):
    Exp = mybir.ActivationFunctionType.Exp

    # ---- persistent SBUF pools (ctx closes before TileContext exits) ----
    pers = ctx.enter_context(tc.tile_pool(name="pers", bufs=1))
    # v_aug padded to 128 cols/head: [v (64) | ones (1) | ones pad (63)].
    # The pad keeps the PV matmuls at full 128-column stationary occupancy,
    # which the PE activity monitor needs to hold the 2.4 GHz clock.
    v_sb = [pers.tile([128, H * 128], BF16, name=f"v{r}", tag=f"v{r}")
            for r in range(KCH)]
    attn_sb = [pers.tile([128, INNER], BF16, name=f"attn{qi}", tag=f"attn{qi}")
               for qi in range(QCH)]
    attnT_sb = [pers.tile([128, QR], BF16, name=f"attnT{m}", tag=f"attnT{m}")
                for m in range(INNER // 128)]
    wo_sb = [pers.tile([128, QD], BF16, name=f"wo{m}", tag=f"wo{m}")
             for m in range(INNER // 128)]
    bias_sb = pers.tile([128, QD], F32, name="bias", tag="bias")
    idf = pers.tile([128, 128], F32, name="idf", tag="idf")
    idb = pers.tile([128, 128], BF16, name="idb", tag="idb")

    make_identity(nc, idf[:])
    make_identity(nc, idb[:])
    nc.sync.dma_start(out=bias_sb[:], in_=bias_d[:])
    for m in range(INNER // 128):
        nc.sync.dma_start(out=wo_sb[m][:], in_=wo_d[m])

    # ---- load pools on the right-side SBUF stack (LIFO close order:
    # ld_a (wv) after h0, ld_c (hsT/wq) after h5, ld_b (encT/wk) after h6)
    ld_b = tc.tile_pool(name="ld_b", bufs=1, side="right")
    ld_c = tc.tile_pool(name="ld_c", bufs=1, side="right")
    ld_a = tc.tile_pool(name="ld_a", bufs=1, side="right")
    pb = ld_b.__enter__()
    pc = ld_c.__enter__()
    pa = ld_a.__enter__()

    encT_sb = [pb.tile([128, SKV], BF16, name=f"encT{j}", tag=f"encT{j}")
               for j in range(CD // 128)]
    wk_sb = [pb.tile([128, INNER], BF16, name=f"wk{j}", tag=f"wk{j}")
             for j in range(CD // 128)]
    wv_sb = [pa.tile([128, INNER], BF16, name=f"wv{j}", tag=f"wv{j}")
             for j in range(CD // 128)]
    hsT_sb = [pc.tile([128, QR], BF16, name=f"hsT{f}", tag=f"hsT{f}")
              for f in range(QD // 128)]
    wq_sb = [pc.tile([128, INNER], BF16, name=f"wq{f}", tag=f"wq{f}")
             for f in range(QD // 128)]

    # load order matters: the first score chunks need encT cols 0:1024 (for
    # kT group 0/1 and vproj r<8), wk, and the q-side - front-load those so
    # the exp stream starts ~12us in instead of after the full 14MB
    for j in range(CD // 128):
        nc.sync.dma_start(out=encT_sb[j][:, 0:1024], in_=encT_d[j][:, 0:1024])
        nc.sync.dma_start(out=wk_sb[j][:], in_=wk_d[j])
        nc.sync.dma_start(out=wv_sb[j][:], in_=wv_d[j])
    for f in range(QD // 128):
        nc.sync.dma_start(out=hsT_sb[f][:], in_=hsT_d[f])
        nc.sync.dma_start(out=wq_sb[f][:], in_=wq_d[f])
    for j in range(CD // 128):
        nc.sync.dma_start(out=encT_sb[j][:, 1024:SKV], in_=encT_d[j][:, 1024:SKV])

    # attention-phase pools.  PSUM budget (8 banks):
    #   spool "S" [128,1024] x2  = 4 banks   (score psums)
    #   pjp  "pj" [128, 512] x2  = 2 banks   (projection psums + normalize tp)
    #   accp "acc"[128,1024] x1  = 2 banks   (PV accumulator)
    att_ctx = ExitStack()
    spool = att_ctx.enter_context(
        tc.tile_pool(name="spool", bufs=2, space="PSUM"))
    pjp = att_ctx.enter_context(
        tc.tile_pool(name="pjp", bufs=2, space="PSUM"))
    accp = att_ctx.enter_context(
        tc.tile_pool(name="accp", bufs=1, space="PSUM"))
    epool = att_ctx.enter_context(tc.tile_pool(name="epool", bufs=6))
    ktp = att_ctx.enter_context(tc.tile_pool(name="ktp", bufs=2))
    qtp = att_ctx.enter_context(tc.tile_pool(name="qtp", bufs=2))
    tup = att_ctx.enter_context(tc.tile_pool(name="tup", bufs=2))
    recp = att_ctx.enter_context(tc.tile_pool(name="recp", bufs=4))
    obp = att_ctx.enter_context(tc.tile_pool(name="obp", bufs=2))

    def kproj_group(kt, m, n):
        ps = pjp.tile([128, 512], F32, name="psk", tag="pj")
        for j in range(CD // 128):
            nc.tensor.matmul(
                ps[:],
                lhsT=wk_sb[j][:, m * 128:(m + 1) * 128],
                rhs=encT_sb[j][:, n * 512:(n + 1) * 512],
                start=(j == 0), stop=(j == CD // 128 - 1),
            )
        nc.vector.tensor_copy(kt[:, n * 512:(n + 1) * 512], ps[:])

    def qproj_group(qt, m, n):
        ps = pjp.tile([128, 512], F32, name="psq", tag="pj")
        for f in range(QD // 128):
            nc.tensor.matmul(
                ps[:],
                lhsT=wq_sb[f][:, m * 128:(m + 1) * 128],
                rhs=hsT_sb[f][:, n * 512:(n + 1) * 512],
                start=(f == 0), stop=(f == QD // 128 - 1),
            )
        nc.vector.tensor_copy(qt[:, n * 512:(n + 1) * 512], ps[:])

    def vproj_group(r):
        nc.gpsimd.memset(v_sb[r][:], 1.0)
        ps = pjp.tile([128, 512], F32, name="psv", tag="pj")
        for j in range(CD // 128):
            nc.tensor.matmul(
                ps[:],
                lhsT=encT_sb[j][:, r * 128:(r + 1) * 128],
                rhs=wv_sb[j][:],
                start=(j == 0), stop=(j == CD // 128 - 1),
            )
        nc.vector.tensor_copy(
            v_sb[r][:].rearrange("p (h d) -> p h d", h=H)[:, :, 0:DH],
            ps[:].rearrange("p (h d) -> p h d", h=H),
        )

    def transpose_group(m, qi):
        # attn [q, inner] -> attnT [inner, q]; ready once heads 2m,2m+1 done
        tb = pjp.tile([128, 128], BF16, name="tb", tag="pj")
        nc.tensor.transpose(
            tb[:], attn_sb[qi][:, m * 128:(m + 1) * 128], idb[:]
        )
        nc.vector.tensor_copy(attnT_sb[m][:, qi * 128:(qi + 1) * 128], tb[:])

    def outproj_group(qi):
        po = pjp.tile([128, QD], F32, name="po", tag="pj")
        for m in range(INNER // 128):
            nc.tensor.matmul(
                po[:],
                lhsT=attnT_sb[m][:, qi * 128:(qi + 1) * 128],
                rhs=wo_sb[m][:],
                start=(m == 0), stop=(m == INNER // 128 - 1),
            )
        ob = obp.tile([128, QD], F32, name="ob", tag="ob")
        nc.vector.tensor_add(ob[:], po[:], bias_sb[:])
        nc.sync.dma_start(out=out_d[qi * 128:(qi + 1) * 128, :], in_=ob[:])

    def norm_chain(h, tu, qi):
        tp = pjp.tile([128, 65], F32, name="tp", tag="pj")
        nc.tensor.transpose(
            tp[:], tu[:, qi * 128:(qi + 1) * 128], idf[0:65, 0:65]
        )
        rec = recp.tile([128, 1], F32, name="rec", tag="rec")
        nc.vector.reciprocal(rec[:], tp[:, DH:DH + 1])
        nc.vector.tensor_scalar_mul(
            attn_sb[qi][:, h * DH:(h + 1) * DH], tp[:, 0:DH], rec[:],
        )

    # PE warm-up: ~35 dependency-free matmuls on the identity tile flip the
    # HAM clock gate to 2.4GHz (~3.4us of sustained activity) before the
    # DMA-gated projection work arrives
    wps = pjp.tile([128, 128], F32, name="wps", tag="pj")
    for _ in range(35):
        nc.tensor.matmul(wps[:], lhsT=idb[:], rhs=idb[:], start=True, stop=True)

    # lead-in for head 0 (runs in the shadow of the remaining DMAs)
    kt = ktp.tile([128, SKV], BF16, name="kt", tag="kt")
    qt = qtp.tile([128, QR], BF16, name="qt", tag="qt")
    kproj_group(kt, 0, 0)
    qproj_group(qt, 0, 0)
    qproj_group(qt, 0, 1)
    kproj_group(kt, 0, 1)
    for r in range(8):
        vproj_group(r)

    # ---- attention ----
    # Per-head interleave schedule (full-util PE work keeps HAM at 2.4GHz):
    #  even head 2m: self-streams kproj(m, n) at r=4(n-1); h0 adds vproj(r)
    #  odd head 2m+1: previous head's deferred normalize chains, qproj for
    #    the next pair, kproj(m+1, 0) late, attn transposes once available
    # The PV stream trails scores/exp by 2 chunks GLOBALLY (across head
    # boundaries) so the exp stream never stalls at a boundary.  The PV
    # accumulator handoff (TU drain + realloc) happens lazily inside the
    # trailing stream.
    pend_norm = []  # deferred normalize thunks for the previous head
    kt_next = qt_next = None
    backlog = []
    acc_state = {"acc": None, "h": None}
    tu_tiles = {}

    def flush_one():
        h, r, e = backlog.pop(0)
        if acc_state["h"] != h:
            if acc_state["acc"] is not None:
                nc.vector.tensor_copy(
                    tu_tiles[acc_state["h"]][:], acc_state["acc"][0:65, :])
            acc_state["acc"] = accp.tile([128, QR], F32, name="acc", tag="acc")
            acc_state["h"] = h
        acc = acc_state["acc"]
        for n in range(QR // 512):
            nc.tensor.matmul(
                acc[:, n * 512:(n + 1) * 512],
                lhsT=v_sb[r][:, h * 128:(h + 1) * 128],
                rhs=e[:, n * 512:(n + 1) * 512],
                start=(r == 0), stop=(r == KCH - 1),
            )

    for h in range(H):
        m, p0 = h // 2, (h % 2) * 64
        even = (h % 2 == 0)
        tu_tiles[h] = tup.tile([65, QR], F32, name="tu", tag="tu")
        sched = {}
        # previous head's deferred normalize chains (r>=3: the TU drain for
        # head h-1 is emitted by the trailing PV stream at r~2)
        for i, thunk in enumerate(pend_norm):
            sched.setdefault(2 * i + 3, []).append(thunk)
        if not even:
            if h < H - 1:
                qt_next = qtp.tile([128, QR], BF16, name="qt", tag="qt")
                kt_next = ktp.tile([128, SKV], BF16, name="kt", tag="kt")
                sched.setdefault(20, []).append(
                    lambda qt_next=qt_next, m=m: qproj_group(qt_next, m + 1, 0))
                sched.setdefault(24, []).append(
                    lambda qt_next=qt_next, m=m: qproj_group(qt_next, m + 1, 1))
                sched.setdefault(28, []).append(
                    lambda kt_next=kt_next, m=m: kproj_group(kt_next, m + 1, 0))
            if m >= 1:
                # attn->attnT transposes for pair m-1 (its norms ran in h-1)
                for i, qi in enumerate(range(QCH)):
                    sched.setdefault(2 * i + 4, []).append(
                        lambda qi=qi, mm=m - 1: transpose_group(mm, qi))
        else:
            for n in range(2 if h == 0 else 1, SKV // 512):
                sched.setdefault(4 * (n - 1), []).append(
                    lambda kt=kt, m=m, n=n: kproj_group(kt, m, n))
            if h == 0:
                for r in range(8, KCH):
                    sched.setdefault(r, []).insert(0, lambda r=r: vproj_group(r))

        for r in range(KCH):
            for thunk in sched.get(r, []):
                thunk()
            S = spool.tile([128, QR], F32, name="S", tag="S")
            for n in range(QR // 512):
                nc.tensor.matmul(
                    S[:, n * 512:(n + 1) * 512],
                    lhsT=kt[p0:p0 + 64, r * 128:(r + 1) * 128],
                    rhs=qt[p0:p0 + 64, n * 512:(n + 1) * 512],
                    start=True, stop=True,
                )
            e = epool.tile([128, QR], BF16, name="E", tag="E")
            nc.scalar.activation(e[:], S[:], Exp)
            backlog.append((h, r, e))
            if len(backlog) > 2:
                flush_one()

        pend_norm = [
            (lambda tu=tu_tiles[h], h=h, qi=qi: norm_chain(h, tu, qi))
            for qi in range(QCH)
        ]
        if not even and h < H - 1:
            kt, qt = kt_next, qt_next
        if h == 0:
            ld_a.__exit__(None, None, None)
        if h == 5:
            ld_c.__exit__(None, None, None)
        if h == 6:
            ld_b.__exit__(None, None, None)

    while backlog:
        flush_one()
    nc.vector.tensor_copy(tu_tiles[H - 1][:], acc_state["acc"][0:65, :])
    for qi in range(QCH):
        norm_chain(H - 1, tu_tiles[H - 1], qi)
        transpose_group(INNER // 128 - 1, qi)
        outproj_group(qi)
    att_ctx.close()


def _build():
    nc = bacc.Bacc("TRN2", target_bir_lowering=False, debug=False,
                   num_devices=NCORES)
    hsT_d = nc.dram_tensor("hsT", [QD // 128, 128, QR], BF16,
                           kind="ExternalInput").ap()
    encT_d = nc.dram_tensor("encT", [CD // 128, 128, SKV], BF16,
                            kind="ExternalInput").ap()
    wq_d = nc.dram_tensor("wq", [QD // 128, 128, INNER], BF16,
                          kind="ExternalInput").ap()
    wk_d = nc.dram_tensor("wk", [CD // 128, 128, INNER], BF16,
                          kind="ExternalInput").ap()
    wv_d = nc.dram_tensor("wv", [CD // 128, 128, INNER], BF16,
                          kind="ExternalInput").ap()
    wo_d = nc.dram_tensor("wo", [INNER // 128, 128, QD], BF16,
                          kind="ExternalInput").ap()
    bias_d = nc.dram_tensor("biasb", [128, QD], F32, kind="ExternalInput").ap()
    out_d = nc.dram_tensor("out", [QR, QD], F32, kind="ExternalOutput").ap()

    from contextlib import ExitStack

    with tile.TileContext(nc) as tc:
        with ExitStack() as ctx:
            _emit(nc, tc, ctx, hsT_d, encT_d, wq_d, wk_d, wv_d, wo_d,
                  bias_d, out_d)
    nc.compile()
    return nc


def _bf16_t_chunks(x32):
    """[R, C] fp32 -> transpose -> bf16 -> [C//128, 128, R]."""
    xt = np.ascontiguousarray(x32.T).astype(NPBF16)
    return xt.reshape(x32.shape[1] // 128, 128, x32.shape[0])


def kernel(hidden_states, encoder_hidden_states, w_q, w_k, w_v, w_out, b_out):
    if "nc" not in _cache:
        _cache["nc"] = _build()
    nc = _cache["nc"]

    hs = np.asarray(hidden_states, np.float32)
    enc = np.asarray(encoder_hidden_states, np.float32)
    wq = (np.asarray(w_q, np.float32) * SCALE).astype(NPBF16)
    wk = np.asarray(w_k, np.float32).astype(NPBF16)
    wv = np.asarray(w_v, np.float32).astype(NPBF16)
    wo = np.asarray(w_out, np.float32).astype(NPBF16)
    bias = np.ascontiguousarray(
        np.broadcast_to(np.asarray(b_out, np.float32), (128, QD))
    )

    wq_c = wq.reshape(QD // 128, 128, INNER)
    wk_c = wk.reshape(CD // 128, 128, INNER)
    wv_c = wv.reshape(CD // 128, 128, INNER)
    wo_c = wo.reshape(INNER // 128, 128, QD)
    encT = [_bf16_t_chunks(enc[b]) for b in range(B)]

    in_maps = []
    for c in range(NCORES):
        b, q0 = c // (NCORES // B), (c % (NCORES // B)) * QR
        in_maps.append({
            "hsT": _bf16_t_chunks(hs[b, q0:q0 + QR, :]),
            "encT": encT[b],
            "wq": wq_c, "wk": wk_c, "wv": wv_c, "wo": wo_c,
            "biasb": bias,
        })

    res = run_bass_kernel_spmd(nc, in_maps, list(range(NCORES)))
    _cache["last_results"] = res

    out = np.empty((B, SQ, QD), np.float32)
    for c in range(NCORES):
        b, q0 = c // (NCORES // B), (c % (NCORES // B)) * QR
        out[b, q0:q0 + QR, :] = res.results[c]["out"]
    return out


# revision 26
# speedup vs baseline: 1.0249x; 1.0249x over previous
"""CrossAttention Trainium2 kernel (8 NeuronCores, SPMD, no collectives).

Problem: nn_CrossAttention_1563368096520
  hidden_states [2, 4096, 512], encoder_hidden_states [2, 4096, 768]
  w_q [512,512], w_k/w_v [768,512], w_out [512,512], b_out [512]
  out = softmax((hs@w_q) @ (enc@w_k)^T * dh^-0.5) @ (enc@w_v) @ w_out + b_out
  (8 heads of dim 64)

Sharding: q-rows. Core c handles batch b=c//4, query rows [(c%4)*1024,
(c%4+1)*1024). Each core recomputes K/V projections for its batch (4x
duplicated) which avoids all cross-core communication.

Layout strategy (bf16 matmuls, fp32 PSUM/softmax-denominator/output):
  - Host pre-transposes hs and enc so the kernel needs no input transposes.
  - qT/kT [inner, rows] come straight out of the projections with w_q/w_k
    as the stationary operand.
  - scores are computed transposed (k on partitions, q free): the exp'd
    score tiles are directly the rhs of the PV matmul - no probs transpose.
  - scores pre-scaled by dh^-0.5 via w_q (exact: *0.125 in bf16).
  - softmax skips max-subtraction (scores in [-2.1, 2.1] for this problem's
    distribution; softmax is shift invariant, exp is exact there).
  - PV runs "flipped": acc[65, q] += v_aug[k,65].T @ expT[k,q] with a ones
    column in v_aug producing the softmax denominator row for free.
    512 N=512 matmuls instead of 2048 N=65 ones.
  - Per head the acc is PE-transposed back to q-partition layout where the
    denominator is a column: reciprocal + tensor_scalar broadcast.
  - K/V/Q projections are interleaved into the first heads' score/exp
    stream so the PE warms up while ACT (the exp bottleneck) streams.
"""

import sys
from contextlib import ExitStack

for _p in ("/opt/trn_rl_repo", "/opt/pypackages"):
    if _p not in sys.path:
        sys.path.append(_p)

import numpy as np
import ml_dtypes

import concourse.bass as bass  # noqa: F401
import concourse.tile as tile
from concourse import bacc, mybir
from concourse.bass_utils import run_bass_kernel_spmd
from concourse.masks import make_identity

BF16 = mybir.dt.bfloat16
F32 = mybir.dt.float32
NPBF16 = ml_dtypes.bfloat16

B, SQ, SKV = 2, 4096, 4096
QD, CD = 512, 768
H, DH = 8, 64
INNER = H * DH  # 512
SCALE = DH ** -0.5
NCORES = 8
QR = (B * SQ) // NCORES  # 1024 query rows per core
QCH = QR // 128          # 8 q chunks per core
KCH = SKV // 128         # 32 kv chunks
DHA = DH + 1             # v columns + ones column

_cache: dict = {}


def _emit(nc, tc, ctx, hsT_d, encT_d, wq_d, wk_d, wv_d, wo_d, bias_d, out_d):
    Exp = mybir.ActivationFunctionType.Exp

    # ---- persistent SBUF pools (ctx closes before TileContext exits) ----
    pers = ctx.enter_context(tc.tile_pool(name="pers", bufs=1))
    # v_aug padded to 128 cols/head: [v (64) | ones (1) | ones pad (63)].
    # The pad keeps the PV matmuls at full 128-column stationary occupancy,
    # which the PE activity monitor needs to hold the 2.4 GHz clock.
    v_sb = [pers.tile([128, H * 128], BF16, name=f"v{r}", tag=f"v{r}")
            for r in range(KCH)]
    attn_sb = [pers.tile([128, INNER], BF16, name=f"attn{qi}", tag=f"attn{qi}")
               for qi in range(QCH)]
    attnT_sb = [pers.tile([128, QR], BF16, name=f"attnT{m}", tag=f"attnT{m}")
                for m in range(INNER // 128)]
    wo_sb = [pers.tile([128, QD], BF16, name=f"wo{m}", tag=f"wo{m}")
             for m in range(INNER // 128)]
    bias_sb = pers.tile([128, QD], F32, name="bias", tag="bias")
    idf = pers.tile([128, 128], F32, name="idf", tag="idf")
    idb = pers.tile([128, 128], BF16, name="idb", tag="idb")

    make_identity(nc, idf[:])
    make_identity(nc, idb[:])
    nc.sync.dma_start(out=bias_sb[:], in_=bias_d[:])
    for m in range(INNER // 128):
        nc.sync.dma_start(out=wo_sb[m][:], in_=wo_d[m])

    # ---- load pools on the right-side SBUF stack (LIFO close order:
    # ld_a (wv) after h0, ld_c (hsT/wq) after h5, ld_b (encT/wk) after h6)
    ld_b = tc.tile_pool(name="ld_b", bufs=1, side="right")
    ld_c = tc.tile_pool(name="ld_c", bufs=1, side="right")
    ld_a = tc.tile_pool(name="ld_a", bufs=1, side="right")
    pb = ld_b.__enter__()
    pc = ld_c.__enter__()
    pa = ld_a.__enter__()

    encT_sb = [pb.tile([128, SKV], BF16, name=f"encT{j}", tag=f"encT{j}")
               for j in range(CD // 128)]
    wk_sb = [pb.tile([128, INNER], BF16, name=f"wk{j}", tag=f"wk{j}")
             for j in range(CD // 128)]
    wv_sb = [pa.tile([128, INNER], BF16, name=f"wv{j}", tag=f"wv{j}")
             for j in range(CD // 128)]
    hsT_sb = [pc.tile([128, QR], BF16, name=f"hsT{f}", tag=f"hsT{f}")
              for f in range(QD // 128)]
    wq_sb = [pc.tile([128, INNER], BF16, name=f"wq{f}", tag=f"wq{f}")
             for f in range(QD // 128)]

    # load order matters: the first score chunks need encT cols 0:1024 (for
    # kT group 0/1 and vproj r<8), wk, and the q-side - front-load those so
    # the exp stream starts ~12us in instead of after the full 14MB
    for j in range(CD // 128):
        nc.sync.dma_start(out=encT_sb[j][:, 0:512], in_=encT_d[j][:, 0:512])
        nc.sync.dma_start(out=wk_sb[j][:], in_=wk_d[j])
    for f in range(QD // 128):
        nc.sync.dma_start(out=hsT_sb[f][:], in_=hsT_d[f])
        nc.sync.dma_start(out=wq_sb[f][:], in_=wq_d[f])
    for j in range(CD // 128):
        nc.sync.dma_start(out=encT_sb[j][:, 512:1024], in_=encT_d[j][:, 512:1024])
        nc.sync.dma_start(out=wv_sb[j][:], in_=wv_d[j])
    for j in range(CD // 128):
        nc.sync.dma_start(out=encT_sb[j][:, 1024:SKV], in_=encT_d[j][:, 1024:SKV])

    # attention-phase pools.  PSUM budget (8 banks):
    #   spool "S" [128,1024] x2  = 4 banks   (score psums)
    #   pjp  "pj" [128, 512] x2  = 2 banks   (projection psums + normalize tp)
    #   accp "acc"[128,1024] x1  = 2 banks   (PV accumulator)
    att_ctx = ExitStack()
    spool = att_ctx.enter_context(
        tc.tile_pool(name="spool", bufs=2, space="PSUM"))
    pjp = att_ctx.enter_context(
        tc.tile_pool(name="pjp", bufs=2, space="PSUM"))
    accp = att_ctx.enter_context(
        tc.tile_pool(name="accp", bufs=1, space="PSUM"))
    epool = att_ctx.enter_context(tc.tile_pool(name="epool", bufs=6))
    ktp = att_ctx.enter_context(tc.tile_pool(name="ktp", bufs=2))
    qtp = att_ctx.enter_context(tc.tile_pool(name="qtp", bufs=2))
    tup = att_ctx.enter_context(tc.tile_pool(name="tup", bufs=2))
    recp = att_ctx.enter_context(tc.tile_pool(name="recp", bufs=4))
    obp = att_ctx.enter_context(tc.tile_pool(name="obp", bufs=2))

    def kproj_part(kt, m, n, part, state):
        # 6-matmul contraction smeared over 3 emissions (2 MMs each) so the
        # score/exp stream never sees a >0.5us PE insertion
        if part == 0:
            state["ps"] = pjp.tile([128, 512], F32, name="psk", tag="pj")
        ps = state["ps"]
        for j in (2 * part, 2 * part + 1):
            nc.tensor.matmul(
                ps[:],
                lhsT=wk_sb[j][:, m * 128:(m + 1) * 128],
                rhs=encT_sb[j][:, n * 512:(n + 1) * 512],
                start=(j == 0), stop=(j == CD // 128 - 1),
            )
        if part == 2:
            nc.vector.tensor_copy(kt[:, n * 512:(n + 1) * 512], ps[:])

    def kproj_group(kt, m, n):
        state = {}
        for part in range(3):
            kproj_part(kt, m, n, part, state)

    def qproj_group(qt, m, n):
        ps = pjp.tile([128, 512], F32, name="psq", tag="pj")
        for f in range(QD // 128):
            nc.tensor.matmul(
                ps[:],
                lhsT=wq_sb[f][:, m * 128:(m + 1) * 128],
                rhs=hsT_sb[f][:, n * 512:(n + 1) * 512],
                start=(f == 0), stop=(f == QD // 128 - 1),
            )
        nc.vector.tensor_copy(qt[:, n * 512:(n + 1) * 512], ps[:])

    def vproj_group(r):
        nc.gpsimd.memset(v_sb[r][:], 1.0)
        ps = pjp.tile([128, 512], F32, name="psv", tag="pj")
        for j in range(CD // 128):
            nc.tensor.matmul(
                ps[:],
                lhsT=encT_sb[j][:, r * 128:(r + 1) * 128],
                rhs=wv_sb[j][:],
                start=(j == 0), stop=(j == CD // 128 - 1),
            )
        nc.vector.tensor_copy(
            v_sb[r][:].rearrange("p (h d) -> p h d", h=H)[:, :, 0:DH],
            ps[:].rearrange("p (h d) -> p h d", h=H),
        )

    def transpose_group(m, qi):
        # attn [q, inner] -> attnT [inner, q]; ready once heads 2m,2m+1 done
        tb = pjp.tile([128, 128], BF16, name="tb", tag="pj")
        nc.tensor.transpose(
            tb[:], attn_sb[qi][:, m * 128:(m + 1) * 128], idb[:]
        )
        nc.vector.tensor_copy(attnT_sb[m][:, qi * 128:(qi + 1) * 128], tb[:])

    def outproj_group(qi):
        po = spool.tile([128, QD], F32, name="po", tag="S")
        for m in range(INNER // 128):
            nc.tensor.matmul(
                po[:],
                lhsT=attnT_sb[m][:, qi * 128:(qi + 1) * 128],
                rhs=wo_sb[m][:],
                start=(m == 0), stop=(m == INNER // 128 - 1),
            )
        ob = obp.tile([128, QD], F32, name="ob", tag="ob")
        nc.vector.tensor_add(ob[:], po[:], bias_sb[:])
        nc.sync.dma_start(out=out_d[qi * 128:(qi + 1) * 128, :], in_=ob[:])

    def norm_chain(h, tu, qi):
        tp = pjp.tile([128, 65], F32, name="tp", tag="pj")
        nc.tensor.transpose(
            tp[:], tu[:, qi * 128:(qi + 1) * 128], idf[0:65, 0:65]
        )
        rec = recp.tile([128, 1], F32, name="rec", tag="rec")
        nc.vector.reciprocal(rec[:], tp[:, DH:DH + 1])
        nc.vector.tensor_scalar_mul(
            attn_sb[qi][:, h * DH:(h + 1) * DH], tp[:, 0:DH], rec[:],
        )

    # PE warm-up: ~35 dependency-free matmuls on the identity tile flip the
    # HAM clock gate to 2.4GHz (~3.4us of sustained activity) before the
    # DMA-gated projection work arrives
    wps = pjp.tile([128, 128], F32, name="wps", tag="pj")
    for _ in range(100):
        nc.tensor.matmul(wps[:], lhsT=idb[:], rhs=idb[:], start=True, stop=True)

    # lead-in for head 0 (runs in the shadow of the remaining DMAs)
    kt = ktp.tile([128, SKV], BF16, name="kt", tag="kt")
    qt = qtp.tile([128, QR], BF16, name="qt", tag="qt")
    kproj_group(kt, 0, 0)
    qproj_group(qt, 0, 0)
    qproj_group(qt, 0, 1)
    kproj_group(kt, 0, 1)
    for r in range(8):
        vproj_group(r)

    # ---- attention ----
    # Per-head interleave schedule (full-util PE work keeps HAM at 2.4GHz):
    #  even head 2m: self-streams kproj(m, n) at r=4(n-1); h0 adds vproj(r)
    #  odd head 2m+1: previous head's deferred normalize chains, qproj for
    #    the next pair, kproj(m+1, 0) late, attn transposes once available
    # The PV stream trails scores/exp by 2 chunks GLOBALLY (across head
    # boundaries) so the exp stream never stalls at a boundary.  The PV
    # accumulator handoff (TU drain + realloc) happens lazily inside the
    # trailing stream.
    pend_norm = []  # deferred normalize thunks for the previous head
    kt_next = qt_next = None
    backlog = []
    acc_state = {"acc": None, "h": None}
    tu_tiles = {}

    def flush_one():
        h, r, e = backlog.pop(0)
        if acc_state["h"] != h:
            if acc_state["acc"] is not None:
                nc.vector.tensor_copy(
                    tu_tiles[acc_state["h"]][:], acc_state["acc"][0:65, :])
            acc_state["acc"] = accp.tile([128, QR], F32, name="acc", tag="acc")
            acc_state["h"] = h
        acc = acc_state["acc"]
        for n in range(QR // 512):
            nc.tensor.matmul(
                acc[:, n * 512:(n + 1) * 512],
                lhsT=v_sb[r][:, h * 128:(h + 1) * 128],
                rhs=e[:, n * 512:(n + 1) * 512],
                start=(r == 0), stop=(r == KCH - 1),
            )

    for h in range(H):
        m, p0 = h // 2, (h % 2) * 64
        even = (h % 2 == 0)
        tu_tiles[h] = tup.tile([65, QR], F32, name="tu", tag="tu")
        sched = {}
        # previous head's deferred normalize chains (r>=3: the TU drain for
        # head h-1 is emitted by the trailing PV stream at r~2)
        for i, thunk in enumerate(pend_norm):
            sched.setdefault(2 * i + 3, []).append(thunk)
        if not even:
            if h < H - 1:
                qt_next = qtp.tile([128, QR], BF16, name="qt", tag="qt")
                kt_next = ktp.tile([128, SKV], BF16, name="kt", tag="kt")
                sched.setdefault(20, []).append(
                    lambda qt_next=qt_next, m=m: qproj_group(qt_next, m + 1, 0))
                sched.setdefault(24, []).append(
                    lambda qt_next=qt_next, m=m: qproj_group(qt_next, m + 1, 1))
                sched.setdefault(28, []).append(
                    lambda kt_next=kt_next, m=m: kproj_group(kt_next, m + 1, 0))
            if m >= 1:
                # attn->attnT transposes for pair m-1 (its norms ran in h-1)
                for i, qi in enumerate(range(QCH)):
                    sched.setdefault(2 * i + 4, []).append(
                        lambda qi=qi, mm=m - 1: transpose_group(mm, qi))
        else:
            for n in range(2 if h == 0 else 1, SKV // 512):
                if h == 0:
                    sched.setdefault(4 * (n - 1), []).append(
                        lambda kt=kt, m=m, n=n: kproj_group(kt, m, n))
                else:
                    state = {}
                    for part in range(3):
                        sched.setdefault(4 * (n - 1) + part, []).append(
                            lambda kt=kt, m=m, n=n, part=part, state=state:
                            kproj_part(kt, m, n, part, state))
            if h == 0:
                for r in range(8, KCH):
                    sched.setdefault(r, []).insert(0, lambda r=r: vproj_group(r))

        for r in range(KCH):
            for thunk in sched.get(r, []):
                thunk()
            S = spool.tile([128, QR], F32, name="S", tag="S")
            for n in range(QR // 512):
                nc.tensor.matmul(
                    S[:, n * 512:(n + 1) * 512],
                    lhsT=kt[p0:p0 + 64, r * 128:(r + 1) * 128],
                    rhs=qt[p0:p0 + 64, n * 512:(n + 1) * 512],
                    start=True, stop=True,
                )
            e = epool.tile([128, QR], BF16, name="E", tag="E")
            nc.scalar.activation(e[:], S[:], Exp)
            backlog.append((h, r, e))
            if len(backlog) > 2:
                flush_one()

        pend_norm = [
            (lambda tu=tu_tiles[h], h=h, qi=qi: norm_chain(h, tu, qi))
            for qi in range(QCH)
        ]
        if not even and h < H - 1:
            kt, qt = kt_next, qt_next
        if h == 0:
            ld_a.__exit__(None, None, None)
        if h == 5:
            ld_c.__exit__(None, None, None)
        if h == 6:
            ld_b.__exit__(None, None, None)

    while backlog:
        flush_one()
    nc.vector.tensor_copy(tu_tiles[H - 1][:], acc_state["acc"][0:65, :])
    for qi in range(QCH):
        norm_chain(H - 1, tu_tiles[H - 1], qi)
        transpose_group(INNER // 128 - 1, qi)
    for qi in range(QCH):
        outproj_group(qi)
    att_ctx.close()


def _build():
    nc = bacc.Bacc("TRN2", target_bir_lowering=False, debug=False,
                   num_devices=NCORES)
    hsT_d = nc.dram_tensor("hsT", [QD // 128, 128, QR], BF16,
                           kind="ExternalInput").ap()
    encT_d = nc.dram_tensor("encT", [CD // 128, 128, SKV], BF16,
                            kind="ExternalInput").ap()
    wq_d = nc.dram_tensor("wq", [QD // 128, 128, INNER], BF16,
                          kind="ExternalInput").ap()
    wk_d = nc.dram_tensor("wk", [CD // 128, 128, INNER], BF16,
                          kind="ExternalInput").ap()
    wv_d = nc.dram_tensor("wv", [CD // 128, 128, INNER], BF16,
                          kind="ExternalInput").ap()
    wo_d = nc.dram_tensor("wo", [INNER // 128, 128, QD], BF16,
                          kind="ExternalInput").ap()
    bias_d = nc.dram_tensor("biasb", [128, QD], F32, kind="ExternalInput").ap()
    out_d = nc.dram_tensor("out", [QR, QD], F32, kind="ExternalOutput").ap()

    from contextlib import ExitStack

    with tile.TileContext(nc) as tc:
        with ExitStack() as ctx:
            _emit(nc, tc, ctx, hsT_d, encT_d, wq_d, wk_d, wv_d, wo_d,
                  bias_d, out_d)
    nc.compile()
    return nc


def _bf16_t_chunks(x32):
    """[R, C] fp32 -> transpose -> bf16 -> [C//128, 128, R]."""
    xt = np.ascontiguousarray(x32.T).astype(NPBF16)
    return xt.reshape(x32.shape[1] // 128, 128, x32.shape[0])


def kernel(hidden_states, encoder_hidden_states, w_q, w_k, w_v, w_out, b_out):
    if "nc" not in _cache:
        _cache["nc"] = _build()
    nc = _cache["nc"]

    hs = np.asarray(hidden_states, np.float32)
    enc = np.asarray(encoder_hidden_states, np.float32)
    wq = (np.asarray(w_q, np.float32) * SCALE).astype(NPBF16)
    wk = np.asarray(w_k, np.float32).astype(NPBF16)
    wv = np.asarray(w_v, np.float32).astype(NPBF16)
    wo = np.asarray(w_out, np.float32).astype(NPBF16)
    bias = np.ascontiguousarray(
        np.broadcast_to(np.asarray(b_out, np.float32), (128, QD))
    )

    wq_c = wq.reshape(QD // 128, 128, INNER)
    wk_c = wk.reshape(CD // 128, 128, INNER)
    wv_c = wv.reshape(CD // 128, 128, INNER)
    wo_c = wo.reshape(INNER // 128, 128, QD)
    encT = [_bf16_t_chunks(enc[b]) for b in range(B)]

    in_maps = []
    for c in range(NCORES):
        b, q0 = c // (NCORES // B), (c % (NCORES // B)) * QR
        in_maps.append({
            "hsT": _bf16_t_chunks(hs[b, q0:q0 + QR, :]),
            "encT": encT[b],
            "wq": wq_c, "wk": wk_c, "wv": wv_c, "wo": wo_c,
            "biasb": bias,
        })

    res = run_bass_kernel_spmd(nc, in_maps, list(range(NCORES)))
    _cache["last_results"] = res

    out = np.empty((B, SQ, QD), np.float32)
    for c in range(NCORES):
        b, q0 = c // (NCORES // B), (c % (NCORES // B)) * QR
        out[b, q0:q0 + QR, :] = res.results[c]["out"]
    return out


# revision 27
# speedup vs baseline: 1.0275x; 1.0026x over previous
"""CrossAttention Trainium2 kernel (8 NeuronCores, SPMD, no collectives).

Problem: nn_CrossAttention_1563368096520
  hidden_states [2, 4096, 512], encoder_hidden_states [2, 4096, 768]
  w_q [512,512], w_k/w_v [768,512], w_out [512,512], b_out [512]
  out = softmax((hs@w_q) @ (enc@w_k)^T * dh^-0.5) @ (enc@w_v) @ w_out + b_out
  (8 heads of dim 64)

Sharding: q-rows. Core c handles batch b=c//4, query rows [(c%4)*1024,
(c%4+1)*1024). Each core recomputes K/V projections for its batch (4x
duplicated) which avoids all cross-core communication.

Layout strategy (bf16 matmuls, fp32 PSUM/softmax-denominator/output):
  - Host pre-transposes hs and enc so the kernel needs no input transposes.
  - qT/kT [inner, rows] come straight out of the projections with w_q/w_k
    as the stationary operand.
  - scores are computed transposed (k on partitions, q free): the exp'd
    score tiles are directly the rhs of the PV matmul - no probs transpose.
  - scores pre-scaled by dh^-0.5 via w_q (exact: *0.125 in bf16).
  - softmax skips max-subtraction (scores in [-2.1, 2.1] for this problem's
    distribution; softmax is shift invariant, exp is exact there).
  - PV runs "flipped": acc[65, q] += v_aug[k,65].T @ expT[k,q] with a ones
    column in v_aug producing the softmax denominator row for free.
    512 N=512 matmuls instead of 2048 N=65 ones.
  - Per head the acc is PE-transposed back to q-partition layout where the
    denominator is a column: reciprocal + tensor_scalar broadcast.
  - K/V/Q projections are interleaved into the first heads' score/exp
    stream so the PE warms up while ACT (the exp bottleneck) streams.
"""

import sys
from contextlib import ExitStack

for _p in ("/opt/trn_rl_repo", "/opt/pypackages"):
    if _p not in sys.path:
        sys.path.append(_p)

import numpy as np
import ml_dtypes

import concourse.bass as bass  # noqa: F401
import concourse.tile as tile
from concourse import bacc, mybir
from concourse.bass_utils import run_bass_kernel_spmd
from concourse.masks import make_identity

BF16 = mybir.dt.bfloat16
F32 = mybir.dt.float32
NPBF16 = ml_dtypes.bfloat16

B, SQ, SKV = 2, 4096, 4096
QD, CD = 512, 768
H, DH = 8, 64
INNER = H * DH  # 512
SCALE = DH ** -0.5
NCORES = 8
QR = (B * SQ) // NCORES  # 1024 query rows per core
QCH = QR // 128          # 8 q chunks per core
KCH = SKV // 128         # 32 kv chunks
DHA = DH + 1             # v columns + ones column

_cache: dict = {}


def _emit(nc, tc, ctx, hsT_d, encT_d, wq_d, wk_d, wv_d, wo_d, bias_d, out_d):
    Exp = mybir.ActivationFunctionType.Exp

    # ---- persistent SBUF pools (ctx closes before TileContext exits) ----
    pers = ctx.enter_context(tc.tile_pool(name="pers", bufs=1))
    # v_aug padded to 128 cols/head: [v (64) | ones (1) | ones pad (63)].
    # The pad keeps the PV matmuls at full 128-column stationary occupancy,
    # which the PE activity monitor needs to hold the 2.4 GHz clock.
    v_sb = [pers.tile([128, H * 128], BF16, name=f"v{r}", tag=f"v{r}")
            for r in range(KCH)]
    attn_sb = [pers.tile([128, INNER], BF16, name=f"attn{qi}", tag=f"attn{qi}")
               for qi in range(QCH)]
    attnT_sb = [pers.tile([128, QR], BF16, name=f"attnT{m}", tag=f"attnT{m}")
                for m in range(INNER // 128)]
    wo_sb = [pers.tile([128, QD], BF16, name=f"wo{m}", tag=f"wo{m}")
             for m in range(INNER // 128)]
    bias_sb = pers.tile([128, QD], F32, name="bias", tag="bias")
    idf = pers.tile([128, 128], F32, name="idf", tag="idf")
    idb = pers.tile([128, 128], BF16, name="idb", tag="idb")

    make_identity(nc, idf[:])
    make_identity(nc, idb[:])
    nc.sync.dma_start(out=bias_sb[:], in_=bias_d[:])
    for m in range(INNER // 128):
        nc.sync.dma_start(out=wo_sb[m][:], in_=wo_d[m])

    # ---- load pools on the right-side SBUF stack (LIFO close order:
    # ld_a (wv) after h0, ld_c (hsT/wq) after h5, ld_b (encT/wk) after h6)
    ld_b = tc.tile_pool(name="ld_b", bufs=1, side="right")
    ld_c = tc.tile_pool(name="ld_c", bufs=1, side="right")
    ld_a = tc.tile_pool(name="ld_a", bufs=1, side="right")
    pb = ld_b.__enter__()
    pc = ld_c.__enter__()
    pa = ld_a.__enter__()

    encT_sb = [pb.tile([128, SKV], BF16, name=f"encT{j}", tag=f"encT{j}")
               for j in range(CD // 128)]
    wk_sb = [pb.tile([128, INNER], BF16, name=f"wk{j}", tag=f"wk{j}")
             for j in range(CD // 128)]
    wv_sb = [pa.tile([128, INNER], BF16, name=f"wv{j}", tag=f"wv{j}")
             for j in range(CD // 128)]
    hsT_sb = [pc.tile([128, QR], BF16, name=f"hsT{f}", tag=f"hsT{f}")
              for f in range(QD // 128)]
    wq_sb = [pc.tile([128, INNER], BF16, name=f"wq{f}", tag=f"wq{f}")
             for f in range(QD // 128)]

    # load order matters: the first score chunks need encT cols 0:1024 (for
    # kT group 0/1 and vproj r<8), wk, and the q-side - front-load those so
    # the exp stream starts ~12us in instead of after the full 14MB
    for j in range(CD // 128):
        nc.sync.dma_start(out=encT_sb[j][:, 0:512], in_=encT_d[j][:, 0:512])
        nc.sync.dma_start(out=wk_sb[j][:], in_=wk_d[j])
    for f in range(QD // 128):
        nc.sync.dma_start(out=hsT_sb[f][:], in_=hsT_d[f])
        nc.sync.dma_start(out=wq_sb[f][:], in_=wq_d[f])
    for j in range(CD // 128):
        nc.sync.dma_start(out=encT_sb[j][:, 512:1024], in_=encT_d[j][:, 512:1024])
        nc.sync.dma_start(out=wv_sb[j][:], in_=wv_d[j])
    for j in range(CD // 128):
        nc.sync.dma_start(out=encT_sb[j][:, 1024:SKV], in_=encT_d[j][:, 1024:SKV])

    # attention-phase pools.  PSUM budget (8 banks):
    #   spool "S" [128,1024] x2  = 4 banks   (score psums)
    #   pjp  "pj" [128, 512] x2  = 2 banks   (projection psums + normalize tp)
    #   accp "acc"[128,1024] x1  = 2 banks   (PV accumulator)
    att_ctx = ExitStack()
    spool = att_ctx.enter_context(
        tc.tile_pool(name="spool", bufs=2, space="PSUM"))
    pjp = att_ctx.enter_context(
        tc.tile_pool(name="pjp", bufs=2, space="PSUM"))
    accp = att_ctx.enter_context(
        tc.tile_pool(name="accp", bufs=1, space="PSUM"))
    epool = att_ctx.enter_context(tc.tile_pool(name="epool", bufs=6))
    ktp = att_ctx.enter_context(tc.tile_pool(name="ktp", bufs=2))
    qtp = att_ctx.enter_context(tc.tile_pool(name="qtp", bufs=2))
    tup = att_ctx.enter_context(tc.tile_pool(name="tup", bufs=2))
    recp = att_ctx.enter_context(tc.tile_pool(name="recp", bufs=4))
    obp = att_ctx.enter_context(tc.tile_pool(name="obp", bufs=2))

    def kproj_part(kt, m, n, part, state):
        # 6-matmul contraction smeared over 3 emissions (2 MMs each) so the
        # score/exp stream never sees a >0.5us PE insertion
        if part == 0:
            state["ps"] = pjp.tile([128, 512], F32, name="psk", tag="pj")
        ps = state["ps"]
        for j in (2 * part, 2 * part + 1):
            nc.tensor.matmul(
                ps[:],
                lhsT=wk_sb[j][:, m * 128:(m + 1) * 128],
                rhs=encT_sb[j][:, n * 512:(n + 1) * 512],
                start=(j == 0), stop=(j == CD // 128 - 1),
            )
        if part == 2:
            nc.vector.tensor_copy(kt[:, n * 512:(n + 1) * 512], ps[:])

    def kproj_group(kt, m, n):
        state = {}
        for part in range(3):
            kproj_part(kt, m, n, part, state)

    def qproj_group(qt, m, n):
        ps = pjp.tile([128, 512], F32, name="psq", tag="pj")
        for f in range(QD // 128):
            nc.tensor.matmul(
                ps[:],
                lhsT=wq_sb[f][:, m * 128:(m + 1) * 128],
                rhs=hsT_sb[f][:, n * 512:(n + 1) * 512],
                start=(f == 0), stop=(f == QD // 128 - 1),
            )
        nc.vector.tensor_copy(qt[:, n * 512:(n + 1) * 512], ps[:])

    def vproj_group(r):
        nc.gpsimd.memset(v_sb[r][:], 1.0)
        ps = pjp.tile([128, 512], F32, name="psv", tag="pj")
        for j in range(CD // 128):
            nc.tensor.matmul(
                ps[:],
                lhsT=encT_sb[j][:, r * 128:(r + 1) * 128],
                rhs=wv_sb[j][:],
                start=(j == 0), stop=(j == CD // 128 - 1),
            )
        nc.vector.tensor_copy(
            v_sb[r][:].rearrange("p (h d) -> p h d", h=H)[:, :, 0:DH],
            ps[:].rearrange("p (h d) -> p h d", h=H),
        )

    def transpose_group(m, qi):
        # attn [q, inner] -> attnT [inner, q]; ready once heads 2m,2m+1 done
        tb = pjp.tile([128, 128], BF16, name="tb", tag="pj")
        nc.tensor.transpose(
            tb[:], attn_sb[qi][:, m * 128:(m + 1) * 128], idb[:]
        )
        nc.vector.tensor_copy(attnT_sb[m][:, qi * 128:(qi + 1) * 128], tb[:])

    def outproj_group(qi):
        po = spool.tile([128, QD], F32, name="po", tag="S")
        for m in range(INNER // 128):
            nc.tensor.matmul(
                po[:],
                lhsT=attnT_sb[m][:, qi * 128:(qi + 1) * 128],
                rhs=wo_sb[m][:],
                start=(m == 0), stop=(m == INNER // 128 - 1),
            )
        ob = obp.tile([128, QD], F32, name="ob", tag="ob")
        nc.vector.tensor_add(ob[:], po[:], bias_sb[:])
        nc.sync.dma_start(out=out_d[qi * 128:(qi + 1) * 128, :], in_=ob[:])

    def norm_chain(h, tu, qi):
        tp = pjp.tile([128, 65], F32, name="tp", tag="pj")
        nc.tensor.transpose(
            tp[:], tu[:, qi * 128:(qi + 1) * 128], idf[0:65, 0:65]
        )
        rec = recp.tile([128, 1], F32, name="rec", tag="rec")
        nc.vector.reciprocal(rec[:], tp[:, DH:DH + 1])
        nc.vector.tensor_scalar_mul(
            attn_sb[qi][:, h * DH:(h + 1) * DH], tp[:, 0:DH], rec[:],
        )

    # PE warm-up: ~35 dependency-free matmuls on the identity tile flip the
    # HAM clock gate to 2.4GHz (~3.4us of sustained activity) before the
    # DMA-gated projection work arrives
    wps = pjp.tile([128, 128], F32, name="wps", tag="pj")
    for _ in range(40):
        nc.tensor.matmul(wps[:], lhsT=idb[:], rhs=idb[:], start=True, stop=True)

    # lead-in for head 0 (runs in the shadow of the remaining DMAs)
    kt = ktp.tile([128, SKV], BF16, name="kt", tag="kt")
    qt = qtp.tile([128, QR], BF16, name="qt", tag="qt")
    kproj_group(kt, 0, 0)
    qproj_group(qt, 0, 0)
    qproj_group(qt, 0, 1)
    kproj_group(kt, 0, 1)
    for r in range(8):
        vproj_group(r)

    # ---- attention ----
    # Per-head interleave schedule (full-util PE work keeps HAM at 2.4GHz):
    #  even head 2m: self-streams kproj(m, n) at r=4(n-1); h0 adds vproj(r)
    #  odd head 2m+1: previous head's deferred normalize chains, qproj for
    #    the next pair, kproj(m+1, 0) late, attn transposes once available
    # The PV stream trails scores/exp by 2 chunks GLOBALLY (across head
    # boundaries) so the exp stream never stalls at a boundary.  The PV
    # accumulator handoff (TU drain + realloc) happens lazily inside the
    # trailing stream.
    pend_norm = []  # deferred normalize thunks for the previous head
    kt_next = qt_next = None
    backlog = []
    acc_state = {"acc": None, "h": None}
    tu_tiles = {}

    def flush_one():
        h, r, e = backlog.pop(0)
        if acc_state["h"] != h:
            if acc_state["acc"] is not None:
                nc.vector.tensor_copy(
                    tu_tiles[acc_state["h"]][:], acc_state["acc"][0:65, :])
            acc_state["acc"] = accp.tile([128, QR], F32, name="acc", tag="acc")
            acc_state["h"] = h
        acc = acc_state["acc"]
        for n in range(QR // 512):
            nc.tensor.matmul(
                acc[:, n * 512:(n + 1) * 512],
                lhsT=v_sb[r][:, h * 128:(h + 1) * 128],
                rhs=e[:, n * 512:(n + 1) * 512],
                start=(r == 0), stop=(r == KCH - 1),
            )

    for h in range(H):
        m, p0 = h // 2, (h % 2) * 64
        even = (h % 2 == 0)
        tu_tiles[h] = tup.tile([65, QR], F32, name="tu", tag="tu")
        sched = {}
        # previous head's deferred normalize chains (r>=3: the TU drain for
        # head h-1 is emitted by the trailing PV stream at r~2)
        for i, thunk in enumerate(pend_norm):
            sched.setdefault(2 * i + 3, []).append(thunk)
        if not even:
            if h < H - 1:
                qt_next = qtp.tile([128, QR], BF16, name="qt", tag="qt")
                kt_next = ktp.tile([128, SKV], BF16, name="kt", tag="kt")
                sched.setdefault(20, []).append(
                    lambda qt_next=qt_next, m=m: qproj_group(qt_next, m + 1, 0))
                sched.setdefault(24, []).append(
                    lambda qt_next=qt_next, m=m: qproj_group(qt_next, m + 1, 1))
                sched.setdefault(28, []).append(
                    lambda kt_next=kt_next, m=m: kproj_group(kt_next, m + 1, 0))
            if m >= 1:
                # attn->attnT transposes for pair m-1 (its norms ran in h-1)
                for i, qi in enumerate(range(QCH)):
                    sched.setdefault(2 * i + 4, []).append(
                        lambda qi=qi, mm=m - 1: transpose_group(mm, qi))
        else:
            for n in range(2 if h == 0 else 1, SKV // 512):
                if h == 0:
                    sched.setdefault(4 * (n - 1), []).append(
                        lambda kt=kt, m=m, n=n: kproj_group(kt, m, n))
                else:
                    state = {}
                    for part in range(3):
                        sched.setdefault(4 * (n - 1) + part, []).append(
                            lambda kt=kt, m=m, n=n, part=part, state=state:
                            kproj_part(kt, m, n, part, state))
            if h == 0:
                for r in range(8, KCH):
                    sched.setdefault(r, []).insert(0, lambda r=r: vproj_group(r))

        for r in range(KCH):
            for thunk in sched.get(r, []):
                thunk()
            S = spool.tile([128, QR], F32, name="S", tag="S")
            for n in range(QR // 512):
                nc.tensor.matmul(
                    S[:, n * 512:(n + 1) * 512],
                    lhsT=kt[p0:p0 + 64, r * 128:(r + 1) * 128],
                    rhs=qt[p0:p0 + 64, n * 512:(n + 1) * 512],
                    start=True, stop=True,
                )
            e = epool.tile([128, QR], BF16, name="E", tag="E")
            nc.scalar.activation(e[:], S[:], Exp)
            backlog.append((h, r, e))
            if len(backlog) > 2:
                flush_one()

        pend_norm = [
            (lambda tu=tu_tiles[h], h=h, qi=qi: norm_chain(h, tu, qi))
            for qi in range(QCH)
        ]
        if not even and h < H - 1:
            kt, qt = kt_next, qt_next
        if h == 0:
            ld_a.__exit__(None, None, None)
        if h == 5:
            ld_c.__exit__(None, None, None)
        if h == 6:
            ld_b.__exit__(None, None, None)

    while backlog:
        flush_one()
    nc.vector.tensor_copy(tu_tiles[H - 1][:], acc_state["acc"][0:65, :])
    for qi in range(QCH):
        norm_chain(H - 1, tu_tiles[H - 1], qi)
        transpose_group(INNER // 128 - 1, qi)
    for qi in range(QCH):
        outproj_group(qi)
    att_ctx.close()


def _build():
    nc = bacc.Bacc("TRN2", target_bir_lowering=False, debug=False,
                   num_devices=NCORES)
    hsT_d = nc.dram_tensor("hsT", [QD // 128, 128, QR], BF16,
                           kind="ExternalInput").ap()
    encT_d = nc.dram_tensor("encT", [CD // 128, 128, SKV], BF16,
                            kind="ExternalInput").ap()
    wq_d = nc.dram_tensor("wq", [QD // 128, 128, INNER], BF16,
                          kind="ExternalInput").ap()
    wk_d = nc.dram_tensor("wk", [CD // 128, 128, INNER], BF16,
                          kind="ExternalInput").ap()
    wv_d = nc.dram_tensor("wv", [CD // 128, 128, INNER], BF16,
                          kind="ExternalInput").ap()
    wo_d = nc.dram_tensor("wo", [INNER // 128, 128, QD], BF16,
                          kind="ExternalInput").ap()
    bias_d = nc.dram_tensor("biasb", [128, QD], F32, kind="ExternalInput").ap()
    out_d = nc.dram_tensor("out", [QR, QD], F32, kind="ExternalOutput").ap()

    from contextlib import ExitStack

    with tile.TileContext(nc) as tc:
        with ExitStack() as ctx:
            _emit(nc, tc, ctx, hsT_d, encT_d, wq_d, wk_d, wv_d, wo_d,
                  bias_d, out_d)
    nc.compile()
    return nc


def _bf16_t_chunks(x32):
    """[R, C] fp32 -> transpose -> bf16 -> [C//128, 128, R]."""
    xt = np.ascontiguousarray(x32.T).astype(NPBF16)
    return xt.reshape(x32.shape[1] // 128, 128, x32.shape[0])


def kernel(hidden_states, encoder_hidden_states, w_q, w_k, w_v, w_out, b_out):
    if "nc" not in _cache:
        _cache["nc"] = _build()
    nc = _cache["nc"]

    hs = np.asarray(hidden_states, np.float32)
    enc = np.asarray(encoder_hidden_states, np.float32)
    wq = (np.asarray(w_q, np.float32) * SCALE).astype(NPBF16)
    wk = np.asarray(w_k, np.float32).astype(NPBF16)
    wv = np.asarray(w_v, np.float32).astype(NPBF16)
    wo = np.asarray(w_out, np.float32).astype(NPBF16)
    bias = np.ascontiguousarray(
        np.broadcast_to(np.asarray(b_out, np.float32), (128, QD))
    )

    wq_c = wq.reshape(QD // 128, 128, INNER)
    wk_c = wk.reshape(CD // 128, 128, INNER)
    wv_c = wv.reshape(CD // 128, 128, INNER)
    wo_c = wo.reshape(INNER // 128, 128, QD)
    encT = [_bf16_t_chunks(enc[b]) for b in range(B)]

    in_maps = []
    for c in range(NCORES):
        b, q0 = c // (NCORES // B), (c % (NCORES // B)) * QR
        in_maps.append({
            "hsT": _bf16_t_chunks(hs[b, q0:q0 + QR, :]),
            "encT": encT[b],
            "wq": wq_c, "wk": wk_c, "wv": wv_c, "wo": wo_c,
            "biasb": bias,
        })

    res = run_bass_kernel_spmd(nc, in_maps, list(range(NCORES)))
    _cache["last_results"] = res

    out = np.empty((B, SQ, QD), np.float32)
    for c in range(NCORES):
        b, q0 = c // (NCORES // B), (c % (NCORES // B)) * QR
        out[b, q0:q0 + QR, :] = res.results[c]["out"]
    return out
